# revision 7
# baseline (speedup 1.0000x reference)
"""Multi-head causal attention kernel for Trainium2 (8 NeuronCores).

Problem: B=4, S=2048, HID=1024, H=16 heads (head_dim 64), causal mask,
fp32 I/O.  out = softmax(mask + (XqWq)(XkWk)^T/8) (XvWv) Wo

Sharding: 8 cores = 4 batches x 2 head-groups.  Core c handles batch
c//2 and heads (c%2)*8 .. +8 (dk slice of 512).  Each core computes a
full-shape [S, HID] partial output (its head-group's contribution
through Wo); the host sums the two partials per batch.

Per-core dataflow (all matmuls in float32r = TF32-like, full PE rate):
  1. PE-transpose X chunks -> X^T, project to qT/kT ([e,s] layout,
     2 heads per 128-partition tile) and v (natural [s,e] layout with a
     ones column appended per head -> PV matmul also produces softmax
     denominators for free).
  2. Attention in transposed [k,q] orientation: logits^T blocks via
     kT-chunk (stationary) x qT (moving); exp on ScalarE straight out of
     PSUM (scale 1/8 folded into Wq host-side); triangular mask applied
     only to diagonal 128x128 blocks; PV matmuls accumulate ctx^T in
     PSUM with causally-restricted column ranges (PSUM has_written
     semantics make partial-range accumulation correct).
  3. Per (head, q-tile): reciprocal of the denominator row, broadcast
     across 64 partitions with a K=1 matmul, multiply-evacuate ctx^T.
  4. Output projection ctx^T.T @ Wo accumulated over dk chunks.
"""

import numpy as np

B, S, HID = 4, 2048, 1024
H_LOCAL, E_LOCAL = 8, 512  # heads / dk columns handled per core
N_CORES = 8
USE_F32R = True

_cached = {}


def _build():
    from concourse import bacc
    import concourse.bass as bass
    import concourse.mybir as mybir
    import concourse.tile as tile
    from concourse.masks import make_identity

    F32 = mybir.dt.float32
    F32R = mybir.dt.float32r if USE_F32R else mybir.dt.float32
    Exp = mybir.ActivationFunctionType.Exp

    nc = bacc.Bacc()
    xq = nc.dram_tensor("xq", [S, HID], F32, kind="ExternalInput")
    xk = nc.dram_tensor("xk", [S, HID], F32, kind="ExternalInput")
    xv = nc.dram_tensor("xv", [S, HID], F32, kind="ExternalInput")
    wq = nc.dram_tensor("wq", [HID, E_LOCAL], F32R, kind="ExternalInput")
    wk = nc.dram_tensor("wk", [HID, E_LOCAL], F32R, kind="ExternalInput")
    wv = nc.dram_tensor("wv", [HID, E_LOCAL], F32R, kind="ExternalInput")
    wo = nc.dram_tensor("wo", [E_LOCAL, HID], F32R, kind="ExternalInput")
    out = nc.dram_tensor("out", [S, HID], F32, kind="ExternalOutput")

    NST = 8          # s-tiles for projection phase
    STW = S // NST   # 256 rows per s-tile
    NSC = STW // 128  # 2 s-chunks per s-tile
    NDC = HID // 128  # 8 d-chunks
    NEC = E_LOCAL // 128  # 4 e-chunks (= head pairs)
    NKC = S // 128   # 16 k-chunks
    NQT = 4          # q-tiles of 512
    NQC = S // 128   # 16 q-chunks for output projection

    with tile.TileContext(nc) as tc:
        with (
            tc.sbuf_pool(name="consts", bufs=1) as consts,
            tc.sbuf_pool(name="persist", bufs=1) as persist,
        ):
            ident = consts.tile([128, 128], F32)
            make_identity(nc, ident)
            # additive causal mask for diagonal blocks in [k, q]
            # orientation: 0 where k <= q, else -1e9 (applied to logits
            # in PSUM before exp).
            trimask = consts.tile([128, 128], F32)
            nc.gpsimd.memset(trimask, 0.0)
            nc.gpsimd.affine_select(
                out=trimask, in_=trimask,
                compare_op=mybir.AluOpType.is_ge, fill=-1e9, base=0,
                pattern=[[1, 128]], channel_multiplier=-1,
            )
            ones64_f32 = consts.tile([1, 64], F32)
            nc.vector.memset(ones64_f32, 1.0)
            ones64 = consts.tile([1, 64], F32R)
            nc.vector.tensor_copy(ones64, ones64_f32)
            ones_col = consts.tile([128, 1], F32)
            nc.vector.memset(ones_col, 1.0)

            # persistent activations
            qt_sb = [persist.tile([128, S], F32R, name=f"qt{i}", tag=f"qt{i}")
                     for i in range(NEC)]
            kt_sb = [persist.tile([128, S], F32R, name=f"kt{i}", tag=f"kt{i}")
                     for i in range(NEC)]
            v_sb = [persist.tile([128, H_LOCAL, 65], F32R, name=f"v{i}",
                                 tag=f"v{i}") for i in range(NKC)]

            # ---------------- projection phase ----------------
            with (
                tc.sbuf_pool(name="proj", bufs=1) as pr,
                tc.psum_pool(name="pp", bufs=1) as pp,
            ):
                wq_sb = pr.tile([128, NDC, E_LOCAL], F32R, tag="wq", bufs=1)
                wk_sb = pr.tile([128, NDC, E_LOCAL], F32R, tag="wk", bufs=1)
                wv_sb = pr.tile([128, NDC, E_LOCAL], F32R, tag="wv", bufs=1)
                nc.sync.dma_start(
                    out=wq_sb, in_=wq.rearrange("(dc p) e -> p dc e", p=128))
                nc.sync.dma_start(
                    out=wk_sb, in_=wk.rearrange("(dc p) e -> p dc e", p=128))
                nc.sync.dma_start(
                    out=wv_sb, in_=wv.rearrange("(dc p) e -> p dc e", p=128))

                for st in range(NST):
                    s0 = st * STW
                    for tname, xdram in (("q", xq), ("k", xk), ("v", xv)):
                        xnat = pr.tile([128, NSC, HID], F32, tag="xnat",
                                       bufs=2, name=f"xnat_{tname}{st}")
                        nc.sync.dma_start(
                            out=xnat,
                            in_=xdram[s0:s0 + STW, :].rearrange(
                                "(sc p) d -> p sc d", p=128))
                        xt = pr.tile([128, NDC, STW], F32R, tag="xt",
                                     bufs=2, name=f"xt_{tname}{st}")
                        for dcp in range(NDC // 2):
                            tp = pp.tile([128, 512], F32, tag="tp", bufs=2,
                                         name=f"tp_{tname}{st}_{dcp}")
                            for k2 in range(2):
                                dc = dcp * 2 + k2
                                for sc in range(NSC):
                                    nc.tensor.transpose(
                                        tp[:, k2 * STW + sc * 128:
                                           k2 * STW + (sc + 1) * 128],
                                        xnat[:, sc, dc * 128:(dc + 1) * 128],
                                        ident)
                            nc.vector.tensor_copy(
                                xt[:, dcp * 2:dcp * 2 + 2, :], tp)

                        if tname in ("q", "k"):
                            wsb = wq_sb if tname == "q" else wk_sb
                            dst = qt_sb if tname == "q" else kt_sb
                            for ec in range(NEC):
                                pj = pp.tile([128, STW], F32, tag="pj",
                                             bufs=3,
                                             name=f"pj_{tname}{st}_{ec}")
                                for dc in range(NDC):
                                    nc.tensor.matmul(
                                        pj,
                                        wsb[:, dc, ec * 128:(ec + 1) * 128],
                                        xt[:, dc, :],
                                        start=(dc == 0), stop=(dc == NDC - 1))
                                nc.vector.tensor_copy(
                                    dst[ec][:, s0:s0 + STW], pj)
                        else:
                            for sc in range(NSC):
                                pv = pp.tile([128, E_LOCAL], F32, tag="pj",
                                             bufs=3, name=f"pv_{st}_{sc}")
                                for dc in range(NDC):
                                    nc.tensor.matmul(
                                        pv,
                                        xt[:, dc, sc * 128:(sc + 1) * 128],
                                        wv_sb[:, dc, :],
                                        start=(dc == 0), stop=(dc == NDC - 1))
                                ci = st * NSC + sc
                                nc.vector.tensor_copy(
                                    v_sb[ci][:, :, 0:64],
                                    pv.rearrange("p (h e) -> p h e",
                                                 h=H_LOCAL))
                                ones_b = bass.AP(
                                    tensor=ones_col.tensor,
                                    offset=ones_col.offset,
                                    ap=[ones_col.ap[0], [0, H_LOCAL],
                                        ones_col.ap[1]],
                                )
                                nc.vector.tensor_copy(
                                    v_sb[ci][:, :, 64:65], ones_b)

            # ---------------- attention + output phase ----------------
            with tc.sbuf_pool(name="late", bufs=1) as late:
                ctx_sb = [late.tile([128, S], F32R, name=f"ctx{i}",
                                    tag=f"ctx{i}", bufs=1)
                          for i in range(NEC)]
                wo_sb = late.tile([128, NEC, HID], F32R, tag="wo", bufs=1)
                nc.sync.dma_start(
                    out=wo_sb, in_=wo.rearrange("(dv p) n -> p dv n", p=128))

                with (
                    tc.sbuf_pool(name="att", bufs=1) as at,
                    tc.psum_pool(name="pa", bufs=1) as pa,
                ):
                    for hp in range(NEC):  # head pair = e-chunk
                        for j in range(NQT):
                            q0 = j * 512
                            nlast = 4 * j + 3
                            cpx = [pa.tile([65, 512], F32, tag="cpx", bufs=2,
                                           name=f"cpx{hp}_{j}_{hi}")
                                   for hi in range(2)]
                            for c in range(4 * j + 4):
                                lg = pa.tile([128, 1024], F32, tag="lg",
                                             bufs=2, name=f"lg{hp}_{j}_{c}")
                                pt = at.tile([128, 1024], F32R, tag="pt",
                                             bufs=3, name=f"pt{hp}_{j}_{c}")
                                for hi in range(2):
                                    nc.tensor.matmul(
                                        lg[:, hi * 512:(hi + 1) * 512],
                                        kt_sb[hp][hi * 64:(hi + 1) * 64,
                                                  c * 128:(c + 1) * 128],
                                        qt_sb[hp][hi * 64:(hi + 1) * 64,
                                                  q0:q0 + 512],
                                        start=True, stop=True)
                                if c >= 4 * j:
                                    m = c - 4 * j
                                    blk = lg.rearrange(
                                        "p (hh q) -> p hh q", hh=2)[
                                        :, :, m * 128:(m + 1) * 128]
                                    tri_b = bass.AP(
                                        tensor=trimask.tensor,
                                        offset=trimask.offset,
                                        ap=[trimask.ap[0], [0, 2],
                                            trimask.ap[1]],
                                    )
                                    nc.vector.tensor_add(blk, blk, tri_b)
                                nc.scalar.activation(pt, lg, Exp)
                                vo = max(0, c * 128 - q0)
                                for hi in range(2):
                                    nc.tensor.matmul(
                                        cpx[hi][:, vo:512],
                                        v_sb[c][:, hp * 2 + hi, :],
                                        pt[:, hi * 512 + vo:(hi + 1) * 512],
                                        start=(c == 0), stop=(c == nlast))
                            for hi in range(2):
                                recip_f = at.tile([1, 512], F32, tag="recipf",
                                                  bufs=2,
                                                  name=f"rcf{hp}_{j}_{hi}")
                                nc.vector.reciprocal(recip_f,
                                                     cpx[hi][64:65, :])
                                recip = at.tile([1, 512], F32R, tag="recip",
                                                bufs=2,
                                                name=f"rc{hp}_{j}_{hi}")
                                nc.vector.tensor_copy(recip, recip_f)
                                bps = pa.tile([64, 512], F32, tag="bps",
                                              bufs=1, name=f"bps{hp}_{j}_{hi}")
                                nc.tensor.matmul(bps, ones64, recip,
                                                 start=True, stop=True)
                                bsb = at.tile([64, 512], F32, tag="bsb",
                                              bufs=2, name=f"bsb{hp}_{j}_{hi}")
                                nc.scalar.copy(bsb, bps)
                                nc.vector.tensor_mul(
                                    ctx_sb[hp][hi * 64:(hi + 1) * 64,
                                               q0:q0 + 512],
                                    cpx[hi][0:64, :], bsb)

                with (
                    tc.sbuf_pool(name="outp", bufs=1) as op_,
                    tc.psum_pool(name="po", bufs=1) as po_,
                ):
                    for qc in range(NQC):
                        for nh in range(2):
                            po = po_.tile([128, 512], F32, tag="op", bufs=2,
                                          name=f"po{qc}_{nh}")
                            for dvc in range(NEC):
                                nc.tensor.matmul(
                                    po,
                                    ctx_sb[dvc][:, qc * 128:(qc + 1) * 128],
                                    wo_sb[:, dvc, nh * 512:(nh + 1) * 512],
                                    start=(dvc == 0), stop=(dvc == NEC - 1))
                            osb = op_.tile([128, 512], F32, tag="osb", bufs=3,
                                           name=f"osb{qc}_{nh}")
                            nc.vector.tensor_copy(osb, po)
                            nc.sync.dma_start(
                                out=out[qc * 128:(qc + 1) * 128,
                                        nh * 512:(nh + 1) * 512],
                                in_=osb)

    nc.compile()
    return nc


def kernel(queries, keys, values, mask=None, Wq=None, Wk=None, Wv=None,
           Wo=None, **_ignored):
    from concourse.bass_utils import run_bass_kernel_spmd

    if "nc" not in _cached:
        _cached["nc"] = _build()
    nc = _cached["nc"]

    scale = np.float32(0.125)  # (DK//H) ** -0.5, exact power of two
    in_maps = []
    for c in range(N_CORES):
        b, g = divmod(c, 2)
        sl = slice(g * E_LOCAL, (g + 1) * E_LOCAL)
        in_maps.append({
            "xq": np.ascontiguousarray(queries[b], dtype=np.float32),
            "xk": np.ascontiguousarray(keys[b], dtype=np.float32),
            "xv": np.ascontiguousarray(values[b], dtype=np.float32),
            "wq": np.ascontiguousarray(Wq[:, sl] * scale),
            "wk": np.ascontiguousarray(Wk[:, sl]),
            "wv": np.ascontiguousarray(Wv[:, sl]),
            "wo": np.ascontiguousarray(Wo[sl, :]),
        })
    res = run_bass_kernel_spmd(nc, in_maps, core_ids=list(range(N_CORES)))
    outs = res.results
    full = np.empty((B, S, HID), np.float32)
    for b in range(B):
        full[b] = outs[2 * b]["out"] + outs[2 * b + 1]["out"]
    return full


# revision 8
# speedup vs baseline: 1.1463x; 1.1463x over previous
"""Multi-head causal attention kernel for Trainium2 (8 NeuronCores).

Problem: B=4, S=2048, HID=1024, H=16 heads (head_dim 64), causal mask,
fp32 I/O.  out = softmax(mask + (XqWq)(XkWk)^T/8) (XvWv) Wo

Sharding: 8 cores = 4 batches x 2 head-groups.  Core c handles batch
c//2 and heads (c%2)*8 .. +8 (dk slice of 512).  Each core computes a
full-shape [S, HID] partial output (its head-group's contribution
through Wo); the host sums the two partials per batch.

Per-core dataflow (all matmuls in float32r = TF32-like, full PE rate):
  1. PE-transpose X chunks -> X^T, project to qT/kT ([e,s] layout,
     2 heads per 128-partition tile) and v (natural [s,e] layout with a
     ones column appended per head -> PV matmul also produces softmax
     denominators for free).
  2. Attention in transposed [k,q] orientation: logits^T blocks via
     kT-chunk (stationary) x qT (moving); exp on ScalarE straight out of
     PSUM (scale 1/8 folded into Wq host-side); triangular mask applied
     only to diagonal 128x128 blocks; PV matmuls accumulate ctx^T in
     PSUM with causally-restricted column ranges (PSUM has_written
     semantics make partial-range accumulation correct).
  3. Per (head, q-tile): reciprocal of the denominator row, broadcast
     across 64 partitions with a K=1 matmul, multiply-evacuate ctx^T.
  4. Output projection ctx^T.T @ Wo accumulated over dk chunks.
"""

import numpy as np

B, S, HID = 4, 2048, 1024
H_LOCAL, E_LOCAL = 8, 512  # heads / dk columns handled per core
N_CORES = 8
USE_F32R = True

_cached = {}


def _build():
    from concourse import bacc
    import concourse.bass as bass
    import concourse.mybir as mybir
    import concourse.tile as tile
    from concourse.masks import make_identity

    F32 = mybir.dt.float32
    F32R = mybir.dt.float32r if USE_F32R else mybir.dt.float32
    Exp = mybir.ActivationFunctionType.Exp

    nc = bacc.Bacc()
    xq = nc.dram_tensor("xq", [S, HID], F32R, kind="ExternalInput")
    xk = nc.dram_tensor("xk", [S, HID], F32R, kind="ExternalInput")
    xv = nc.dram_tensor("xv", [S, HID], F32R, kind="ExternalInput")
    wq = nc.dram_tensor("wq", [HID, E_LOCAL], F32R, kind="ExternalInput")
    wk = nc.dram_tensor("wk", [HID, E_LOCAL], F32R, kind="ExternalInput")
    wv = nc.dram_tensor("wv", [HID, E_LOCAL], F32R, kind="ExternalInput")
    wo = nc.dram_tensor("wo", [E_LOCAL, HID], F32R, kind="ExternalInput")
    out = nc.dram_tensor("out", [S, HID], F32, kind="ExternalOutput")

    NST = 8          # s-tiles for projection phase
    STW = S // NST   # 256 rows per s-tile
    NSC = STW // 128  # 2 s-chunks per s-tile
    NDC = HID // 128  # 8 d-chunks
    NEC = E_LOCAL // 128  # 4 e-chunks (= head pairs)
    NKC = S // 128   # 16 k-chunks
    NQT = 4          # q-tiles of 512
    NQC = S // 128   # 16 q-chunks for output projection

    with tile.TileContext(nc) as tc:
        with (
            tc.sbuf_pool(name="consts", bufs=1) as consts,
            tc.sbuf_pool(name="persist", bufs=1) as persist,
        ):
            ident_f = consts.tile([128, 128], F32)
            make_identity(nc, ident_f)
            ident = consts.tile([128, 128], F32R)
            nc.vector.tensor_copy(ident, ident_f)
            # additive causal mask for diagonal blocks in [k, q]
            # orientation: 0 where k <= q, else -1e9 (applied to logits
            # in PSUM before exp).
            trimask = consts.tile([128, 128], F32)
            nc.gpsimd.memset(trimask, 0.0)
            nc.gpsimd.affine_select(
                out=trimask, in_=trimask,
                compare_op=mybir.AluOpType.is_ge, fill=-1e9, base=0,
                pattern=[[1, 128]], channel_multiplier=-1,
            )
            ones64_f32 = consts.tile([1, 64], F32)
            nc.vector.memset(ones64_f32, 1.0)
            ones64 = consts.tile([1, 64], F32R)
            nc.vector.tensor_copy(ones64, ones64_f32)
            ones_col = consts.tile([128, 1], F32)
            nc.vector.memset(ones_col, 1.0)

            # persistent activations
            qt_sb = [persist.tile([128, S], F32R, name=f"qt{i}", tag=f"qt{i}")
                     for i in range(NEC)]
            kt_sb = [persist.tile([128, S], F32R, name=f"kt{i}", tag=f"kt{i}")
                     for i in range(NEC)]
            v_sb = [persist.tile([128, H_LOCAL, 65], F32R, name=f"v{i}",
                                 tag=f"v{i}") for i in range(NKC)]

            # ---------------- projection phase ----------------
            with (
                tc.sbuf_pool(name="proj", bufs=1) as pr,
                tc.psum_pool(name="pp", bufs=1) as pp,
            ):
                wq_sb = pr.tile([128, NDC, E_LOCAL], F32R, tag="wq", bufs=1)
                wk_sb = pr.tile([128, NDC, E_LOCAL], F32R, tag="wk", bufs=1)
                wv_sb = pr.tile([128, NDC, E_LOCAL], F32R, tag="wv", bufs=1)
                nc.sync.dma_start(
                    out=wq_sb, in_=wq.rearrange("(dc p) e -> p dc e", p=128))
                nc.sync.dma_start(
                    out=wk_sb, in_=wk.rearrange("(dc p) e -> p dc e", p=128))
                nc.sync.dma_start(
                    out=wv_sb, in_=wv.rearrange("(dc p) e -> p dc e", p=128))

                for st in range(NST):
                    s0 = st * STW
                    for tname, xdram in (("q", xq), ("k", xk), ("v", xv)):
                        xnat = pr.tile([128, NSC, HID], F32R, tag="xnat",
                                       bufs=2, name=f"xnat_{tname}{st}")
                        nc.sync.dma_start(
                            out=xnat,
                            in_=xdram[s0:s0 + STW, :].rearrange(
                                "(sc p) d -> p sc d", p=128))
                        xt = pr.tile([128, NDC, STW], F32R, tag="xt",
                                     bufs=2, name=f"xt_{tname}{st}")
                        for dcp in range(NDC // 2):
                            tp = pp.tile([128, 512], F32R, tag="tp", bufs=2,
                                         name=f"tp_{tname}{st}_{dcp}")
                            for k2 in range(2):
                                dc = dcp * 2 + k2
                                for sc in range(NSC):
                                    nc.tensor.transpose(
                                        tp[:, k2 * STW + sc * 128:
                                           k2 * STW + (sc + 1) * 128],
                                        xnat[:, sc, dc * 128:(dc + 1) * 128],
                                        ident)
                            nc.vector.tensor_copy(
                                xt[:, dcp * 2:dcp * 2 + 2, :], tp)

                        if tname in ("q", "k"):
                            wsb = wq_sb if tname == "q" else wk_sb
                            dst = qt_sb if tname == "q" else kt_sb
                            for ec in range(NEC):
                                pj = pp.tile([128, STW], F32, tag="pj",
                                             bufs=3,
                                             name=f"pj_{tname}{st}_{ec}")
                                for dc in range(NDC):
                                    nc.tensor.matmul(
                                        pj,
                                        wsb[:, dc, ec * 128:(ec + 1) * 128],
                                        xt[:, dc, :],
                                        start=(dc == 0), stop=(dc == NDC - 1))
                                nc.vector.tensor_copy(
                                    dst[ec][:, s0:s0 + STW], pj)
                        else:
                            for sc in range(NSC):
                                pv = pp.tile([128, E_LOCAL], F32, tag="pj",
                                             bufs=3, name=f"pv_{st}_{sc}")
                                for dc in range(NDC):
                                    nc.tensor.matmul(
                                        pv,
                                        xt[:, dc, sc * 128:(sc + 1) * 128],
                                        wv_sb[:, dc, :],
                                        start=(dc == 0), stop=(dc == NDC - 1))
                                ci = st * NSC + sc
                                nc.vector.tensor_copy(
                                    v_sb[ci][:, :, 0:64],
                                    pv.rearrange("p (h e) -> p h e",
                                                 h=H_LOCAL))
                                ones_b = bass.AP(
                                    tensor=ones_col.tensor,
                                    offset=ones_col.offset,
                                    ap=[ones_col.ap[0], [0, H_LOCAL],
                                        ones_col.ap[1]],
                                )
                                nc.vector.tensor_copy(
                                    v_sb[ci][:, :, 64:65], ones_b)

            # ---------------- attention + output phase ----------------
            with tc.sbuf_pool(name="late", bufs=1) as late:
                ctx_sb = [late.tile([128, S], F32R, name=f"ctx{i}",
                                    tag=f"ctx{i}", bufs=1)
                          for i in range(NEC)]
                wo_sb = late.tile([128, NEC, HID], F32R, tag="wo", bufs=1)
                nc.sync.dma_start(
                    out=wo_sb, in_=wo.rearrange("(dv p) n -> p dv n", p=128))

                with (
                    tc.sbuf_pool(name="att", bufs=1) as at,
                    tc.psum_pool(name="pa", bufs=1) as pa,
                ):
                    for j in range(NQT):
                        q0 = j * 512
                        nlast = 4 * j + 3
                        for hp in range(NEC):  # head pair = e-chunk
                            cpx = [pa.tile([65, 512], F32, tag="acc", bufs=4,
                                           name=f"cpx{hp}_{j}_{hi}")
                                   for hi in range(2)]
                            for c in range(4 * j + 4):
                                vo = max(0, c * 128 - q0)
                                lg = pa.tile([128, 1024], F32, tag="lg",
                                             bufs=2, name=f"lg{hp}_{j}_{c}")
                                pt = at.tile([128, 1024], F32R, tag="pt",
                                             bufs=3, name=f"pt{hp}_{j}_{c}")
                                for hi in range(2):
                                    nc.tensor.matmul(
                                        lg[:, hi * 512 + vo:(hi + 1) * 512],
                                        kt_sb[hp][hi * 64:(hi + 1) * 64,
                                                  c * 128:(c + 1) * 128],
                                        qt_sb[hp][hi * 64:(hi + 1) * 64,
                                                  q0 + vo:q0 + 512],
                                        start=True, stop=True)
                                if c >= 4 * j:
                                    m = c - 4 * j
                                    blk = lg.rearrange(
                                        "p (hh q) -> p hh q", hh=2)[
                                        :, :, m * 128:(m + 1) * 128]
                                    tri_b = bass.AP(
                                        tensor=trimask.tensor,
                                        offset=trimask.offset,
                                        ap=[trimask.ap[0], [0, 2],
                                            trimask.ap[1]],
                                    )
                                    nc.vector.tensor_add(blk, blk, tri_b)
                                nc.scalar.activation(pt[:, vo:1024],
                                                     lg[:, vo:1024], Exp)
                                for hi in range(2):
                                    nc.tensor.matmul(
                                        cpx[hi][:, vo:512],
                                        v_sb[c][:, hp * 2 + hi, :],
                                        pt[:, hi * 512 + vo:(hi + 1) * 512],
                                        start=(c == 0), stop=(c == nlast))
                            for hi in range(2):
                                recip_f = at.tile([1, 512], F32, tag="recipf",
                                                  bufs=2,
                                                  name=f"rcf{hp}_{j}_{hi}")
                                nc.vector.reciprocal(recip_f,
                                                     cpx[hi][64:65, :])
                                bcast = at.tile([64, 512], F32, tag="bcast",
                                                bufs=2,
                                                name=f"bc{hp}_{j}_{hi}")
                                nc.gpsimd.partition_broadcast(bcast, recip_f)
                                nc.vector.tensor_mul(
                                    ctx_sb[hp][hi * 64:(hi + 1) * 64,
                                               q0:q0 + 512],
                                    cpx[hi][0:64, :], bcast)
                        # output projection for this q block (all heads done)
                        for qc in range(4 * j, 4 * j + 4):
                            for nh in range(2):
                                po = pa.tile([128, 512], F32, tag="acc",
                                             bufs=4, name=f"po{qc}_{nh}")
                                for dvc in range(NEC):
                                    nc.tensor.matmul(
                                        po,
                                        ctx_sb[dvc][:,
                                                    qc * 128:(qc + 1) * 128],
                                        wo_sb[:, dvc, nh * 512:(nh + 1) * 512],
                                        start=(dvc == 0),
                                        stop=(dvc == NEC - 1))
                                osb = at.tile([128, 512], F32, tag="osb",
                                              bufs=3, name=f"osb{qc}_{nh}")
                                nc.vector.tensor_copy(osb, po)
                                nc.sync.dma_start(
                                    out=out[qc * 128:(qc + 1) * 128,
                                            nh * 512:(nh + 1) * 512],
                                    in_=osb)

    nc.compile()
    return nc


def kernel(queries, keys, values, mask=None, Wq=None, Wk=None, Wv=None,
           Wo=None, **_ignored):
    from concourse.bass_utils import run_bass_kernel_spmd

    if "nc" not in _cached:
        _cached["nc"] = _build()
    nc = _cached["nc"]

    scale = np.float32(0.125)  # (DK//H) ** -0.5, exact power of two
    in_maps = []
    for c in range(N_CORES):
        b, g = divmod(c, 2)
        sl = slice(g * E_LOCAL, (g + 1) * E_LOCAL)
        in_maps.append({
            "xq": np.ascontiguousarray(queries[b], dtype=np.float32),
            "xk": np.ascontiguousarray(keys[b], dtype=np.float32),
            "xv": np.ascontiguousarray(values[b], dtype=np.float32),
            "wq": np.ascontiguousarray(Wq[:, sl] * scale),
            "wk": np.ascontiguousarray(Wk[:, sl]),
            "wv": np.ascontiguousarray(Wv[:, sl]),
            "wo": np.ascontiguousarray(Wo[sl, :]),
        })
    res = run_bass_kernel_spmd(nc, in_maps, core_ids=list(range(N_CORES)))
    outs = res.results
    full = np.empty((B, S, HID), np.float32)
    for b in range(B):
        full[b] = outs[2 * b]["out"] + outs[2 * b + 1]["out"]
    return full
